# revision 42
# baseline (speedup 1.0000x reference)
"""MoDA attention Trainium2 kernel, 8-way head-parallel.

Tuning vs the original baseline (213.1us -> 211.5us measured):
 - psS bufs 3->4 (extra S-matmul/exp pipeline slack), psO 3->2 (the 1-unit
   epilogue delay only needs 2 o_ps banks live).
 - outproj stores alternate between the SP and Activation DMA queues,
   halving store-issue serialization on the sync queue.

Sharding: core c owns Q heads {2c, 2c+1} and K/V head c (their GQA group),
plus that K head's depth caches. Each core computes its heads' projections,
RoPE, joint seq+depth softmax attention, and a partial output projection
(rows 2c*128:(2c+2)*128 of Wo). Host sums the 8 partial outputs.

Device layouts (host-prepared, bf16):
  xT   [DM, T]      x transposed  (rhs / lhsT tiles with D on partitions)
  wq   [DM, 2*HD]   Wq columns for this core's 2 Q heads
  wk   [DM, HD], wv [DM, HD]
  wo   [2, HD, DM]  Wo rows for the 2 heads
  cosT/sinT [HD, T] rope tables transposed
  kdT/vdT [L, HD, T] depth caches transposed (per K head)
  mask [128, 128]   upper-tri keep-mask for causal diagonal blocks

On-device dataflow (per Q head, S^T layout => no P transposes):
  Q^T/K^T = W^T @ x^T via matmul(lhsT=W_tile, rhs=xT_tile), rope'd in
  [d, T] layout; V natural via matmul(lhsT=xT_tile, rhs=wv_tile).
  S^T[j,i] tile = matmul(lhsT=K^T j-block, rhs=Q^T i-chunk); U = exp(S*scale)
  (no max subtraction: |scale*logits| < ~5 for these inputs); causal mask by
  zeroing/masking U; Z row += ones.T @ U; O^T += matmul(lhsT=V_j, rhs=U).
  Depth: eU_l = Q^T * kdT_l elementwise; dl_l = ones.T @ eU_l; wd_l =
  exp(dl*scale); Z += wd_l (K=1 matmul); bcast_l = outer(ones, wd_l);
  O^T += sum_l vdT_l * bcast_l (DVE); O^T *= outer(ones, 1/Z).
  out[128, n] = sum_h matmul(lhsT=O^T_h i-block, rhs=wo_h n-chunk) -> DRAM.
"""

import os
import sys

sys.path.insert(0, "/opt/trn_rl_repo")

import numpy as np
import ml_dtypes

import concourse.bass as bass
import concourse.tile as tile
import concourse.mybir as mybir
from concourse import bacc
from concourse.bass_utils import run_bass_kernel_spmd

BF16 = mybir.dt.bfloat16
FP32 = mybir.dt.float32
NPBF16 = ml_dtypes.bfloat16

HQ, HK, HD, DM = 16, 8, 128, 2048
L = 4
GQA = HQ // HK
SCALE = float(HD) ** -0.5
N_CORES = 8
NQH = 2  # Q heads per core
TC = 512  # T chunk (free dim of most matmuls)
DK = DM // 128  # contraction tiles

_programs = {}
last_result = None


def _ts(i, n):
    return bass.ts(i, n)


def build_program(T):
    nc = bacc.Bacc(
        "TRN2",
        target_bir_lowering=False,
        debug=False,
        enable_asserts=False,
        num_devices=N_CORES,
    )

    xT = nc.dram_tensor("xT", [DM, T], BF16, kind="ExternalInput").ap()
    wq = nc.dram_tensor("wq", [DM, NQH * HD], BF16, kind="ExternalInput").ap()
    wk = nc.dram_tensor("wk", [DM, HD], BF16, kind="ExternalInput").ap()
    wv = nc.dram_tensor("wv", [DM, HD], BF16, kind="ExternalInput").ap()
    wo = nc.dram_tensor("wo", [NQH, HD, DM], BF16, kind="ExternalInput").ap()
    cosT = nc.dram_tensor("cosT", [HD, T], BF16, kind="ExternalInput").ap()
    sinT = nc.dram_tensor("sinT", [HD, T], BF16, kind="ExternalInput").ap()
    kdT = nc.dram_tensor("kdT", [L, HD, T], BF16, kind="ExternalInput").ap()
    vdT = nc.dram_tensor("vdT", [L, HD, T], BF16, kind="ExternalInput").ap()
    mask = nc.dram_tensor("mask", [128, 128], BF16, kind="ExternalInput").ap()
    out = nc.dram_tensor("out", [T, DM], BF16, kind="ExternalOutput").ap()

    NC_CHUNKS = T // TC  # i-chunks per head
    NTB = T // 128  # 128-blocks in T

    with tile.TileContext(nc) as tc:
        with tc.tile_pool(name="const", bufs=1) as cp:
            # ---- persistent SBUF tensors (loads emitted in need-order) ----
            wq_sb = cp.tile([128, DK, NQH * HD], BF16)
            wk_sb = cp.tile([128, DK, HD], BF16)
            wv_sb = cp.tile([128, DK, HD], BF16)
            wo_sb = cp.tile([128, NQH, DM], BF16)
            cos_sb = cp.tile([128, T], BF16)
            sin_sb = cp.tile([128, T], BF16)
            kdT_sb = cp.tile([128, L, T], BF16)
            vdT_sb = cp.tile([128, L, T], BF16)
            mask_sb = cp.tile([128, 128], BF16)
            ones_sb = cp.tile([128, 1], BF16)
            nc.vector.memset(ones_sb[:], 1.0)
            ones1_sb = cp.tile([1, 128], BF16)
            nc.vector.memset(ones1_sb[:], 1.0)

            qT_sb = cp.tile([128, NQH, T], BF16)  # rope'd Q^T per head
            kT_sb = cp.tile([128, T], BF16)  # rope'd K^T
            v_sb = cp.tile([128, NTB, HD], BF16)  # V natural, [i-in-block, block, d]
            oT_sb = cp.tile([128, NQH, T], BF16)  # normalized O^T per head

            # ---- phase A: projections + rope (kt-outer so compute starts
            # after the first 512KB of xT) ----
            with tc.tile_pool(name="psA", bufs=4, space="PSUM") as psA, \
                 tc.tile_pool(name="psAv", bufs=3, space="PSUM") as psAv, \
                 tc.tile_pool(name="cpX", bufs=1) as cpX, \
                 tc.tile_pool(name="sA", bufs=3) as sA:
                xT_sb = cpX.tile([128, DK, T], BF16)
                for cc in range(NC_CHUNKS):
                    nc.sync.dma_start(xT_sb[:, 0, _ts(cc, TC)],
                                      xT[_ts(0, 128), _ts(cc, TC)])
                nc.sync.dma_start(wk_sb[:], wk.rearrange("(kt p) m -> p kt m", p=128))
                nc.sync.dma_start(
                    wq_sb[:], wq.rearrange("(kt p) m -> p kt m", p=128)
                )
                for kt in range(1, DK):
                    nc.sync.dma_start(xT_sb[:, kt, :], xT[_ts(kt, 128), :])
                nc.sync.dma_start(wv_sb[:], wv.rearrange("(kt p) m -> p kt m", p=128))
                nc.sync.dma_start(cos_sb[:], cosT[:])
                nc.sync.dma_start(sin_sb[:], sinT[:])
                for l in range(L):
                    nc.sync.dma_start(kdT_sb[:, l, :], kdT[l])
                for l in range(L):
                    nc.sync.dma_start(vdT_sb[:, l, :], vdT[l])
                nc.sync.dma_start(mask_sb[:], mask[:])
                nc.sync.dma_start(wo_sb[:], wo.rearrange("h p n -> p h n"))

                def rope_chunk(ps, dst, c):
                    cs = cos_sb[:, _ts(c, TC)]
                    sn = sin_sb[:, _ts(c, TC)]
                    praw = sA.tile([128, TC], BF16, tag="praw")
                    nc.scalar.copy(praw[:], ps[:])
                    prot = sA.tile([128, TC], BF16, tag="prot")
                    nc.vector.tensor_copy(prot[0:64, :], praw[64:128, :])
                    nc.vector.tensor_copy(prot[64:128, :], praw[0:64, :])
                    t1 = sA.tile([128, TC], BF16, tag="t1")
                    nc.vector.tensor_mul(t1[:], praw[:], cs)
                    t2 = sA.tile([128, TC], BF16, tag="t2")
                    nc.vector.tensor_mul(t2[:], prot[:], sn)
                    nc.vector.tensor_tensor(
                        dst[0:64, :], t1[0:64, :], t2[0:64, :],
                        op=mybir.AluOpType.subtract,
                    )
                    nc.vector.tensor_add(dst[64:128, :], t1[64:128, :], t2[64:128, :])

                def proj_kt_outer(w_ap, dsts):
                    # dsts: list of (dst slice, chunk idx); one psum per chunk
                    pss = [psA.tile([128, TC], FP32, tag="proj", name=f"ps{i}")
                           for i in range(NC_CHUNKS)]
                    for kt in range(DK):
                        for c in range(NC_CHUNKS):
                            nc.tensor.matmul(
                                pss[c][:], w_ap(kt), xT_sb[:, kt, _ts(c, TC)],
                                start=(kt == 0), stop=(kt == DK - 1),
                            )
                    for c in range(NC_CHUNKS):
                        rope_chunk(pss[c], dsts(c), c)

                proj_kt_outer(lambda kt: wk_sb[:, kt, :],
                              lambda c: kT_sb[:, _ts(c, TC)])
                for h in range(NQH):
                    proj_kt_outer(lambda kt: wq_sb[:, kt, _ts(h, HD)],
                                  lambda c: qT_sb[:, h, _ts(c, TC)])
                for tb in range(NTB):
                    vp = psAv.tile([128, HD], FP32, tag="vp")
                    for kt in range(DK):
                        nc.tensor.matmul(
                            vp[:], xT_sb[:, kt, _ts(tb, 128)], wv_sb[:, kt, :],
                            start=(kt == 0), stop=(kt == DK - 1),
                        )
                    nc.scalar.copy(v_sb[:, tb, :], vp[:])

            # ---- phase B: attention, software-pipelined emission ----
            # For each (chunk, head) unit: emit the PE-heavy part now, the
            # DVE/POOL epilogue one unit later, the chunk out-projection after
            # both heads' epilogues. Keeps PE fed during serial epilogues.
            with tc.tile_pool(name="psS", bufs=4, space="PSUM") as psS, \
                 tc.tile_pool(name="psO", bufs=2, space="PSUM") as psO, \
                 tc.tile_pool(name="psZ", bufs=2, space="PSUM") as psZ, \
                 tc.tile_pool(name="sBc", bufs=10) as sBc, \
                 tc.tile_pool(name="sB", bufs=22) as sB, \
                 tc.tile_pool(name="sEu", bufs=5) as sEu, \
                 tc.tile_pool(name="sB2", bufs=2) as sB2, \
                 tc.tile_pool(name="sC", bufs=6) as sC, \
                 tc.tile_pool(name="sWd", bufs=10) as sWd:

                def attention_part(c, h):
                    qch = qT_sb[:, h, _ts(c, TC)]
                    o_ps = psO.tile([128, TC], FP32, tag="o")
                    z_ps = psZ.tile([1, TC], FP32, tag="z")
                    jmax = (c + 1) * (TC // 128)
                    c0 = c * (TC // 128)
                    offs = [max(0, (jb - c0)) * 128 for jb in range(jmax)]
                    us = []
                    for jb in range(jmax):
                        off = offs[jb]
                        s_ps = psS.tile([128, TC], FP32, tag="s")
                        nc.tensor.matmul(
                            s_ps[:, off:TC], kT_sb[:, _ts(jb, 128)],
                            qch[:, off:TC], start=True, stop=True,
                        )
                        u = sB.tile([128, TC], BF16, tag="u")
                        nc.scalar.activation(
                            u[:, off:TC], s_ps[:, off:TC],
                            mybir.ActivationFunctionType.Exp, scale=SCALE,
                        )
                        if jb >= c0:
                            nc.vector.tensor_mul(
                                u[:, off : off + 128], u[:, off : off + 128],
                                mask_sb[:],
                            )
                        us.append(u)
                        if jb >= 2:
                            zj = jb - 2
                            nc.tensor.matmul(
                                z_ps[0:1, offs[zj]:TC], ones_sb[:],
                                us[zj][:, offs[zj]:TC],
                                start=(zj == 0), stop=False,
                            )
                    for zj in range(max(0, jmax - 2), jmax):
                        nc.tensor.matmul(
                            z_ps[0:1, offs[zj]:TC], ones_sb[:],
                            us[zj][:, offs[zj]:TC],
                            start=(zj == 0), stop=False,
                        )
                    wd = []
                    for l in range(L):
                        eu = sEu.tile([128, TC], BF16, tag="eu")
                        nc.vector.tensor_mul(
                            eu[:], qch, kdT_sb[:, l, _ts(c, TC)]
                        )
                        dl = psZ.tile([1, TC], FP32, tag="z", name=f"dl{l}")
                        nc.tensor.matmul(
                            dl[:], ones_sb[:], eu[:], start=True, stop=True
                        )
                        wdl = sWd.tile([1, TC], BF16, tag="wd", name=f"wd{l}")
                        nc.scalar.activation(
                            wdl[:], dl[:],
                            mybir.ActivationFunctionType.Exp, scale=SCALE,
                        )
                        wd.append(wdl)
                    for l in range(L):
                        nc.tensor.matmul(
                            z_ps[:], ones1_sb[0:1, 0:1], wd[l][:],
                            start=False, stop=(l == L - 1),
                        )
                    # broadcasts + zinv here: POOL/DVE run them under the
                    # PE matmuls; the delayed epilogue is then pure DVE with
                    # all inputs ready
                    bcs = []
                    for l in range(L):
                        bc = sBc.tile([128, TC], BF16, tag="bc", name=f"bc{l}")
                        nc.gpsimd.partition_broadcast(bc[:], wd[l][0:1, :])
                        bcs.append(bc)
                    zinv = sB2.tile([1, TC], FP32, tag="zi")
                    nc.vector.reciprocal_approx_fast(zinv[:], z_ps[:])
                    zb = sBc.tile([128, TC], FP32, tag="zb")
                    nc.gpsimd.partition_broadcast(zb[:], zinv[0:1, :])
                    for jb in range(jmax):
                        off = offs[jb]
                        nc.tensor.matmul(
                            o_ps[:, off:TC], v_sb[:, jb, :], us[jb][:, off:TC],
                            start=(jb == 0), stop=(jb == jmax - 1),
                        )
                    return o_ps, bcs, zb

                def epilogue_part(c, h, o_ps, bcs, zb):
                    t_acc = sB2.tile([128, TC], BF16, tag="tacc")
                    t_tmp = sB2.tile([128, TC], BF16, tag="ttmp")
                    for l in range(L):
                        dst = t_acc if l == 0 else t_tmp
                        nc.vector.tensor_mul(
                            dst[:], vdT_sb[:, l, _ts(c, TC)], bcs[l][:]
                        )
                        if l > 0:
                            nc.vector.tensor_add(t_acc[:], t_acc[:], t_tmp[:])
                    o_sum = sB2.tile([128, TC], FP32, tag="osum")
                    nc.vector.tensor_add(o_sum[:], o_ps[:], t_acc[:])
                    nc.vector.tensor_mul(
                        oT_sb[:, h, _ts(c, TC)], o_sum[:], zb[:]
                    )

                def outproj_chunk(c, half=None):
                    rng = range(TC // 128)
                    if half is not None:
                        rng = rng[:2] if half == 0 else rng[2:]
                    for tbl in rng:
                        tb = c * (TC // 128) + tbl
                        for nchunk in range(DM // TC):
                            op = psS.tile([128, TC], FP32, tag="s", name="op")
                            for h in range(NQH):
                                nc.tensor.matmul(
                                    op[:], oT_sb[:, h, _ts(tb, 128)],
                                    wo_sb[:, h, _ts(nchunk, TC)],
                                    start=(h == 0), stop=(h == NQH - 1),
                                )
                            res = sC.tile([128, TC], BF16, tag="res")
                            if (tb + nchunk) % 2 == 0:
                                nc.scalar.copy(res[:], op[:])
                            else:
                                nc.vector.tensor_copy(res[:], op[:])
                            eng = (nc.sync, nc.scalar)[nchunk % 2]
                            eng.dma_start(
                                out[_ts(tb, 128), _ts(nchunk, TC)], res[:]
                            )

                units = [(c, h) for c in reversed(range(NC_CHUNKS)) for h in range(NQH)]
                pend_epi = []  # (c, h, o_ps, bcs, zb)
                pend_out = []  # (chunk, next_half)
                for idx, (c, h) in enumerate(units):
                    state = attention_part(c, h)
                    if pend_out:
                        pc2, half = pend_out[0]
                        outproj_chunk(pc2, half)
                        if half == 1:
                            pend_out.pop(0)
                        else:
                            pend_out[0] = (pc2, 1)
                    if pend_epi:
                        pc, ph, po, pbcs, pzb = pend_epi.pop(0)
                        epilogue_part(pc, ph, po, pbcs, pzb)
                        if ph == NQH - 1:
                            pend_out.append((pc, 0))
                    pend_epi.append((c, h) + tuple(state))
                while pend_epi:
                    pc, ph, po, pbcs, pzb = pend_epi.pop(0)
                    epilogue_part(pc, ph, po, pbcs, pzb)
                    if ph == NQH - 1:
                        pend_out.append((pc, 0))
                while pend_out:
                    outproj_chunk(pend_out.pop(0)[0])

    nc.compile()
    return nc


def get_program(T):
    if T not in _programs:
        _programs[T] = build_program(T)
    return _programs[T]


def make_in_maps(x, depth_k, depth_v, cos, sin, Wq, Wk, Wv, Wo, T):
    xT16 = np.ascontiguousarray(x[0].T).astype(NPBF16)
    cosT16 = np.ascontiguousarray(cos[0, 0].T).astype(NPBF16)
    sinT16 = np.ascontiguousarray(sin[0, 0].T).astype(NPBF16)
    mask16 = np.triu(np.ones((128, 128), np.float32)).astype(NPBF16)
    in_maps = []
    for c in range(N_CORES):
        wq_c = np.ascontiguousarray(Wq[:, 2 * c * HD : (2 * c + 2) * HD]).astype(NPBF16)
        wk_c = np.ascontiguousarray(Wk[:, c * HD : (c + 1) * HD]).astype(NPBF16)
        wv_c = np.ascontiguousarray(Wv[:, c * HD : (c + 1) * HD]).astype(NPBF16)
        wo_c = np.ascontiguousarray(
            Wo[2 * c * HD : (2 * c + 2) * HD, :].reshape(NQH, HD, DM)
        ).astype(NPBF16)
        kdT_c = np.ascontiguousarray(depth_k[:, 0, c].transpose(0, 2, 1)).astype(NPBF16)
        vdT_c = np.ascontiguousarray(depth_v[:, 0, c].transpose(0, 2, 1)).astype(NPBF16)
        in_maps.append(
            {
                "xT": xT16, "wq": wq_c, "wk": wk_c, "wv": wv_c, "wo": wo_c,
                "cosT": cosT16, "sinT": sinT16, "kdT": kdT_c, "vdT": vdT_c,
                "mask": mask16,
            }
        )
    return in_maps


def kernel(x, depth_k, depth_v, cos, sin, Wq, Wk, Wv, Wo):
    x = np.asarray(x, np.float32)
    T = x.shape[1]
    nc = get_program(T)
    in_maps = make_in_maps(
        x, np.asarray(depth_k, np.float32), np.asarray(depth_v, np.float32),
        np.asarray(cos, np.float32), np.asarray(sin, np.float32),
        np.asarray(Wq, np.float32), np.asarray(Wk, np.float32),
        np.asarray(Wv, np.float32), np.asarray(Wo, np.float32), T,
    )
    trace = bool(os.environ.get("MODA_TRACE"))
    res = run_bass_kernel_spmd(nc, in_maps, list(range(N_CORES)), trace=trace)
    global last_result
    last_result = res
    total = np.zeros((T, DM), np.float32)
    for c in range(N_CORES):
        total += res.results[c]["out"].astype(np.float32)
    return total.reshape(1, T, DM)



# revision 44
# speedup vs baseline: 1.0472x; 1.0472x over previous
"""MoDA attention Trainium2 kernel, 8-way head-parallel.

Sharding: core c owns Q heads {2c, 2c+1} and K/V head c (their GQA group),
plus that K head's depth caches. Each core computes its heads' projections,
RoPE, joint seq+depth softmax attention, and a partial output projection
(rows 2c*128:(2c+2)*128 of Wo). Host sums the 8 partial outputs.

Device layouts (host-prepared, bf16):
  xT   [DM, T]      x transposed  (rhs / lhsT tiles with D on partitions)
  wq   [DM, 2*HD]   Wq columns for this core's 2 Q heads
  wk   [DM, HD], wv [DM, HD]
  wo   [2, HD, DM]  Wo rows for the 2 heads
  cosT/sinT [HD, T] rope tables transposed
  kdT/vdT [L, HD, T] depth caches transposed (per K head)
  mask [128, 128]   upper-tri keep-mask for causal diagonal blocks

On-device dataflow (per Q head, S^T layout => no P transposes):
  Q^T/K^T = W^T @ x^T via matmul(lhsT=W_tile, rhs=xT_tile), rope'd in
  [d, T] layout; V natural via matmul(lhsT=xT_tile, rhs=wv_tile).
  S^T[j,i] tile = matmul(lhsT=K^T j-block, rhs=Q^T i-chunk); U = exp(S*scale)
  (no max subtraction: |scale*logits| < ~5 for these inputs); causal mask by
  zeroing/masking U; Z row += ones.T @ U; O^T += matmul(lhsT=V_j, rhs=U).
  Depth: eU_l = Q^T * kdT_l elementwise; dl_l = ones.T @ eU_l; wd_l =
  exp(dl*scale); Z += wd_l (K=1 matmul); bcast_l = outer(ones, wd_l);
  O^T += sum_l vdT_l * bcast_l (DVE); O^T *= outer(ones, 1/Z).
  out[128, n] = sum_h matmul(lhsT=O^T_h i-block, rhs=wo_h n-chunk) -> DRAM.
"""

import os
import sys

sys.path.insert(0, "/opt/trn_rl_repo")

import numpy as np
import ml_dtypes

import concourse.bass as bass
import concourse.tile as tile
import concourse.mybir as mybir
from concourse import bacc
from concourse.bass_utils import run_bass_kernel_spmd

BF16 = mybir.dt.bfloat16
FP32 = mybir.dt.float32
NPBF16 = ml_dtypes.bfloat16

HQ, HK, HD, DM = 16, 8, 128, 2048
L = 4
GQA = HQ // HK
SCALE = float(HD) ** -0.5
N_CORES = 8
NQH = 2  # Q heads per core
TC = 512  # T chunk (free dim of most matmuls)
DK = DM // 128  # contraction tiles

_programs = {}
last_result = None


def _ts(i, n):
    return bass.ts(i, n)


def build_program(T):
    nc = bacc.Bacc(
        "TRN2",
        target_bir_lowering=False,
        debug=False,
        enable_asserts=False,
        num_devices=N_CORES,
    )

    xT = nc.dram_tensor("xT", [DM, T], BF16, kind="ExternalInput").ap()
    wq = nc.dram_tensor("wq", [DM, NQH * HD], BF16, kind="ExternalInput").ap()
    wk = nc.dram_tensor("wk", [DM, HD], BF16, kind="ExternalInput").ap()
    wv = nc.dram_tensor("wv", [DM, HD], BF16, kind="ExternalInput").ap()
    wo = nc.dram_tensor("wo", [NQH, HD, DM], BF16, kind="ExternalInput").ap()
    cosT = nc.dram_tensor("cosT", [HD, T], BF16, kind="ExternalInput").ap()
    sinT = nc.dram_tensor("sinT", [HD, T], BF16, kind="ExternalInput").ap()
    kdT = nc.dram_tensor("kdT", [L, HD, T], BF16, kind="ExternalInput").ap()
    vdT = nc.dram_tensor("vdT", [L, HD, T], BF16, kind="ExternalInput").ap()
    mask = nc.dram_tensor("mask", [128, 128], BF16, kind="ExternalInput").ap()
    out = nc.dram_tensor("out", [T, DM], BF16, kind="ExternalOutput").ap()

    NC_CHUNKS = T // TC  # i-chunks per head
    NTB = T // 128  # 128-blocks in T

    with tile.TileContext(nc) as tc:
        with tc.tile_pool(name="const", bufs=1) as cp:
            # ---- persistent SBUF tensors (loads emitted in need-order) ----
            wq_sb = cp.tile([128, DK, NQH * HD], BF16)
            wk_sb = cp.tile([128, DK, HD], BF16)
            wv_sb = cp.tile([128, DK, HD], BF16)
            wo_sb = cp.tile([128, NQH, DM], BF16)
            cos_sb = cp.tile([128, T], BF16)
            sin_sb = cp.tile([128, T], BF16)
            kdT_sb = cp.tile([128, L, T], BF16)
            vdT_sb = cp.tile([128, L, T], BF16)
            mask_sb = cp.tile([128, 128], BF16)
            ones_sb = cp.tile([128, 1], BF16)
            nc.vector.memset(ones_sb[:], 1.0)
            ones1_sb = cp.tile([1, 128], BF16)
            nc.vector.memset(ones1_sb[:], 1.0)

            qT_sb = cp.tile([128, NQH, T], BF16)  # rope'd Q^T per head
            kT_sb = cp.tile([128, T], BF16)  # rope'd K^T
            v_sb = cp.tile([128, NTB, HD], BF16)  # V natural, [i-in-block, block, d]
            oT_sb = cp.tile([128, NQH, T], BF16)  # normalized O^T per head

            # ---- phase A: projections + rope (kt-outer so compute starts
            # after the first 512KB of xT) ----
            with tc.tile_pool(name="psA", bufs=4, space="PSUM") as psA, \
                 tc.tile_pool(name="psAv", bufs=3, space="PSUM") as psAv, \
                 tc.tile_pool(name="cpX", bufs=1) as cpX, \
                 tc.tile_pool(name="sA", bufs=3) as sA:
                xT_sb = cpX.tile([128, DK, T], BF16)
                for cc in range(NC_CHUNKS):
                    nc.sync.dma_start(xT_sb[:, 0, _ts(cc, TC)],
                                      xT[_ts(0, 128), _ts(cc, TC)])
                nc.sync.dma_start(wk_sb[:], wk.rearrange("(kt p) m -> p kt m", p=128))
                nc.sync.dma_start(
                    wq_sb[:], wq.rearrange("(kt p) m -> p kt m", p=128)
                )
                for kt in range(1, DK):
                    nc.sync.dma_start(xT_sb[:, kt, :], xT[_ts(kt, 128), :])
                nc.sync.dma_start(wv_sb[:], wv.rearrange("(kt p) m -> p kt m", p=128))
                nc.sync.dma_start(cos_sb[:], cosT[:])
                nc.sync.dma_start(sin_sb[:], sinT[:])
                for l in range(L):
                    nc.sync.dma_start(kdT_sb[:, l, :], kdT[l])
                for l in range(L):
                    nc.sync.dma_start(vdT_sb[:, l, :], vdT[l])
                nc.sync.dma_start(mask_sb[:], mask[:])
                nc.sync.dma_start(wo_sb[:], wo.rearrange("h p n -> p h n"))

                def rope_chunk(ps, dst, c):
                    cs = cos_sb[:, _ts(c, TC)]
                    sn = sin_sb[:, _ts(c, TC)]
                    praw = sA.tile([128, TC], BF16, tag="praw")
                    nc.scalar.copy(praw[:], ps[:])
                    prot = sA.tile([128, TC], BF16, tag="prot")
                    nc.vector.tensor_copy(prot[0:64, :], praw[64:128, :])
                    nc.vector.tensor_copy(prot[64:128, :], praw[0:64, :])
                    t1 = sA.tile([128, TC], BF16, tag="t1")
                    nc.vector.tensor_mul(t1[:], praw[:], cs)
                    t2 = sA.tile([128, TC], BF16, tag="t2")
                    nc.vector.tensor_mul(t2[:], prot[:], sn)
                    nc.vector.tensor_tensor(
                        dst[0:64, :], t1[0:64, :], t2[0:64, :],
                        op=mybir.AluOpType.subtract,
                    )
                    nc.vector.tensor_add(dst[64:128, :], t1[64:128, :], t2[64:128, :])

                def proj_kt_outer(w_ap, dsts):
                    # dsts: list of (dst slice, chunk idx); one psum per chunk
                    pss = [psA.tile([128, TC], FP32, tag="proj", name=f"ps{i}")
                           for i in range(NC_CHUNKS)]
                    for kt in range(DK):
                        for c in range(NC_CHUNKS):
                            nc.tensor.matmul(
                                pss[c][:], w_ap(kt), xT_sb[:, kt, _ts(c, TC)],
                                start=(kt == 0), stop=(kt == DK - 1),
                            )
                    for c in range(NC_CHUNKS):
                        rope_chunk(pss[c], dsts(c), c)

                proj_kt_outer(lambda kt: wk_sb[:, kt, :],
                              lambda c: kT_sb[:, _ts(c, TC)])
                for h in range(NQH):
                    proj_kt_outer(lambda kt: wq_sb[:, kt, _ts(h, HD)],
                                  lambda c: qT_sb[:, h, _ts(c, TC)])
                for tb in range(NTB):
                    vp = psAv.tile([128, HD], FP32, tag="vp")
                    for kt in range(DK):
                        nc.tensor.matmul(
                            vp[:], xT_sb[:, kt, _ts(tb, 128)], wv_sb[:, kt, :],
                            start=(kt == 0), stop=(kt == DK - 1),
                        )
                    nc.scalar.copy(v_sb[:, tb, :], vp[:])

            # ---- phase B: attention, software-pipelined emission ----
            # For each (chunk, head) unit: emit the PE-heavy part now, the
            # DVE/POOL epilogue one unit later, the chunk out-projection after
            # both heads' epilogues. Keeps PE fed during serial epilogues.
            with tc.tile_pool(name="psS", bufs=4, space="PSUM") as psS, \
                 tc.tile_pool(name="psO", bufs=2, space="PSUM") as psO, \
                 tc.tile_pool(name="psZ", bufs=2, space="PSUM") as psZ, \
                 tc.tile_pool(name="sBc", bufs=10) as sBc, \
                 tc.tile_pool(name="sB", bufs=30) as sB, \
                 tc.tile_pool(name="sEu", bufs=5) as sEu, \
                 tc.tile_pool(name="sB2", bufs=2) as sB2, \
                 tc.tile_pool(name="sC", bufs=8) as sC, \
                 tc.tile_pool(name="sWd", bufs=10) as sWd:

                def attention_part(c, h):
                    qch = qT_sb[:, h, _ts(c, TC)]
                    o_ps = psO.tile([128, TC], FP32, tag="o")
                    z_ps = psZ.tile([1, TC], FP32, tag="z")
                    jmax = (c + 1) * (TC // 128)
                    c0 = c * (TC // 128)
                    offs = [max(0, (jb - c0)) * 128 for jb in range(jmax)]
                    us = []
                    for jb in range(jmax):
                        off = offs[jb]
                        s_ps = psS.tile([128, TC], FP32, tag="s")
                        nc.tensor.matmul(
                            s_ps[:, off:TC], kT_sb[:, _ts(jb, 128)],
                            qch[:, off:TC], start=True, stop=True,
                        )
                        u = sB.tile([128, TC], BF16, tag="u")
                        nc.scalar.activation(
                            u[:, off:TC], s_ps[:, off:TC],
                            mybir.ActivationFunctionType.Exp, scale=SCALE,
                        )
                        if jb >= c0:
                            nc.vector.tensor_mul(
                                u[:, off : off + 128], u[:, off : off + 128],
                                mask_sb[:],
                            )
                        us.append(u)
                    wd = []
                    for l in range(L):
                        eu = sEu.tile([128, TC], BF16, tag="eu")
                        nc.vector.tensor_mul(
                            eu[:], qch, kdT_sb[:, l, _ts(c, TC)]
                        )
                        dl = psZ.tile([1, TC], FP32, tag="z", name=f"dl{l}")
                        nc.tensor.matmul(
                            dl[:], ones_sb[:], eu[:], start=True, stop=True
                        )
                        wdl = sWd.tile([1, TC], BF16, tag="wd", name=f"wd{l}")
                        nc.scalar.activation(
                            wdl[:], dl[:],
                            mybir.ActivationFunctionType.Exp, scale=SCALE,
                        )
                        wd.append(wdl)
                    for jb in range(jmax):
                        off = offs[jb]
                        nc.tensor.matmul(
                            z_ps[0:1, off:TC], ones_sb[:], us[jb][:, off:TC],
                            start=(jb == 0), stop=False,
                        )
                    for l in range(L):
                        nc.tensor.matmul(
                            z_ps[:], ones1_sb[0:1, 0:1], wd[l][:],
                            start=False, stop=(l == L - 1),
                        )
                    # broadcasts + zinv here: POOL/DVE run them under the
                    # PE matmuls; the delayed epilogue is then pure DVE with
                    # all inputs ready
                    bcs = []
                    for l in range(L):
                        bc = sBc.tile([128, TC], BF16, tag="bc", name=f"bc{l}")
                        nc.gpsimd.partition_broadcast(bc[:], wd[l][0:1, :])
                        bcs.append(bc)
                    zinv = sB2.tile([1, TC], FP32, tag="zi")
                    nc.vector.reciprocal_approx_fast(zinv[:], z_ps[:])
                    zb = sBc.tile([128, TC], FP32, tag="zb")
                    nc.gpsimd.partition_broadcast(zb[:], zinv[0:1, :])
                    for jb in range(jmax):
                        off = offs[jb]
                        nc.tensor.matmul(
                            o_ps[:, off:TC], v_sb[:, jb, :], us[jb][:, off:TC],
                            start=(jb == 0), stop=(jb == jmax - 1),
                        )
                    return o_ps, bcs, zb

                def epilogue_part(c, h, o_ps, bcs, zb):
                    t_acc = sB2.tile([128, TC], BF16, tag="tacc")
                    t_tmp = sB2.tile([128, TC], BF16, tag="ttmp")
                    for l in range(L):
                        dst = t_acc if l == 0 else t_tmp
                        nc.vector.tensor_mul(
                            dst[:], vdT_sb[:, l, _ts(c, TC)], bcs[l][:]
                        )
                        if l > 0:
                            nc.vector.tensor_add(t_acc[:], t_acc[:], t_tmp[:])
                    o_sum = sB2.tile([128, TC], FP32, tag="osum")
                    nc.vector.tensor_add(o_sum[:], o_ps[:], t_acc[:])
                    nc.vector.tensor_mul(
                        oT_sb[:, h, _ts(c, TC)], o_sum[:], zb[:]
                    )

                def outproj_chunk(c, half=None):
                    rng = range(TC // 128)
                    if half is not None:
                        rng = rng[:2] if half == 0 else rng[2:]
                    for tbl in rng:
                        tb = c * (TC // 128) + tbl
                        for nchunk in range(DM // TC):
                            op = psS.tile([128, TC], FP32, tag="s", name="op")
                            for h in range(NQH):
                                nc.tensor.matmul(
                                    op[:], oT_sb[:, h, _ts(tb, 128)],
                                    wo_sb[:, h, _ts(nchunk, TC)],
                                    start=(h == 0), stop=(h == NQH - 1),
                                )
                            res = sC.tile([128, TC], BF16, tag="res")
                            if (tb + nchunk) % 2 == 0:
                                nc.scalar.copy(res[:], op[:])
                            else:
                                nc.vector.tensor_copy(res[:], op[:])
                            eng = (nc.sync, nc.scalar)[nchunk % 2]
                            eng.dma_start(
                                out[_ts(tb, 128), _ts(nchunk, TC)], res[:]
                            )

                units = [(c, h) for c in reversed(range(NC_CHUNKS)) for h in range(NQH)]
                pend_epi = []  # (c, h, o_ps, bcs, zb)
                pend_out = []  # (chunk, next_half)
                for idx, (c, h) in enumerate(units):
                    state = attention_part(c, h)
                    if pend_out:
                        pc2, half = pend_out[0]
                        outproj_chunk(pc2, half)
                        if half == 1:
                            pend_out.pop(0)
                        else:
                            pend_out[0] = (pc2, 1)
                    if pend_epi:
                        pc, ph, po, pbcs, pzb = pend_epi.pop(0)
                        epilogue_part(pc, ph, po, pbcs, pzb)
                        if ph == NQH - 1:
                            pend_out.append((pc, 0))
                    pend_epi.append((c, h) + tuple(state))
                while pend_epi:
                    pc, ph, po, pbcs, pzb = pend_epi.pop(0)
                    epilogue_part(pc, ph, po, pbcs, pzb)
                    if ph == NQH - 1:
                        pend_out.append((pc, 0))
                while pend_out:
                    outproj_chunk(pend_out.pop(0)[0])

    nc.compile()
    return nc


def get_program(T):
    if T not in _programs:
        _programs[T] = build_program(T)
    return _programs[T]


def make_in_maps(x, depth_k, depth_v, cos, sin, Wq, Wk, Wv, Wo, T):
    xT16 = np.ascontiguousarray(x[0].T).astype(NPBF16)
    cosT16 = np.ascontiguousarray(cos[0, 0].T).astype(NPBF16)
    sinT16 = np.ascontiguousarray(sin[0, 0].T).astype(NPBF16)
    mask16 = np.triu(np.ones((128, 128), np.float32)).astype(NPBF16)
    in_maps = []
    for c in range(N_CORES):
        wq_c = np.ascontiguousarray(Wq[:, 2 * c * HD : (2 * c + 2) * HD]).astype(NPBF16)
        wk_c = np.ascontiguousarray(Wk[:, c * HD : (c + 1) * HD]).astype(NPBF16)
        wv_c = np.ascontiguousarray(Wv[:, c * HD : (c + 1) * HD]).astype(NPBF16)
        wo_c = np.ascontiguousarray(
            Wo[2 * c * HD : (2 * c + 2) * HD, :].reshape(NQH, HD, DM)
        ).astype(NPBF16)
        kdT_c = np.ascontiguousarray(depth_k[:, 0, c].transpose(0, 2, 1)).astype(NPBF16)
        vdT_c = np.ascontiguousarray(depth_v[:, 0, c].transpose(0, 2, 1)).astype(NPBF16)
        in_maps.append(
            {
                "xT": xT16, "wq": wq_c, "wk": wk_c, "wv": wv_c, "wo": wo_c,
                "cosT": cosT16, "sinT": sinT16, "kdT": kdT_c, "vdT": vdT_c,
                "mask": mask16,
            }
        )
    return in_maps


def kernel(x, depth_k, depth_v, cos, sin, Wq, Wk, Wv, Wo):
    x = np.asarray(x, np.float32)
    T = x.shape[1]
    nc = get_program(T)
    in_maps = make_in_maps(
        x, np.asarray(depth_k, np.float32), np.asarray(depth_v, np.float32),
        np.asarray(cos, np.float32), np.asarray(sin, np.float32),
        np.asarray(Wq, np.float32), np.asarray(Wk, np.float32),
        np.asarray(Wv, np.float32), np.asarray(Wo, np.float32), T,
    )
    trace = bool(os.environ.get("MODA_TRACE"))
    res = run_bass_kernel_spmd(nc, in_maps, list(range(N_CORES)), trace=trace)
    global last_result
    last_result = res
    total = np.zeros((T, DM), np.float32)
    for c in range(N_CORES):
        total += res.results[c]["out"].astype(np.float32)
    return total.reshape(1, T, DM)

